# revision 1
# baseline (speedup 1.0000x reference)
"""Multi-head self-attention (B=4,S=2048,D=1024,H=16,DH=64, causal) on 8 trn2 cores.

Sharding: core c -> batch b=c//2, head-group g=c%2 (8 heads each).
Per-core: QKV projections (bf16 matmul, f32 accum), S^T = K@Q^T attention with
ones-column denominator trick, exp on ACT (no max-subtraction: |scores|<~25
safe in f32), renorm via reciprocal+partition_broadcast, output projection
producing the partial out^T. Host sums the two head-group partials per batch.

K-projection quirk (reference views k as (B,S,DH,H)): head h uses Wk rows
[dh*16+h for dh in range(64)] -- handled by host-side row gather.
"""
import numpy as np

import concourse.mybir as mybir
import concourse.tile as tile
from concourse import bacc
from concourse.bass_utils import run_bass_kernel_spmd

F32 = mybir.dt.float32
BF16 = mybir.dt.bfloat16
AF = mybir.ActivationFunctionType

B, S, D, H, DH = 4, 2048, 1024, 16, 64
FG = 512          # features per head-group (8 heads * 64)
N_CORES = 8
SCALE = 0.125     # 1/sqrt(64)

_NC = None


def _build():
    nc = bacc.Bacc("TRN2", target_bir_lowering=False, debug=False,
                   num_devices=N_CORES, enable_asserts=False)
    xT_d = nc.dram_tensor("xT", [D, S], F32, kind="ExternalInput").ap()
    wqT_d = nc.dram_tensor("wqT", [D, FG], F32, kind="ExternalInput").ap()
    wkT_d = nc.dram_tensor("wkT", [D, FG], F32, kind="ExternalInput").ap()
    wvT_d = nc.dram_tensor("wvT", [D, FG], F32, kind="ExternalInput").ap()
    wpT_d = nc.dram_tensor("wpT", [FG, D], F32, kind="ExternalInput").ap()
    bqs_d = nc.dram_tensor("bqs", [128, 4], F32, kind="ExternalInput").ap()
    bks_d = nc.dram_tensor("bks", [128, 4], F32, kind="ExternalInput").ap()
    bvs_d = nc.dram_tensor("bvs", [128, 4], F32, kind="ExternalInput").ap()
    bps_d = nc.dram_tensor("bps", [128, 8], F32, kind="ExternalInput").ap()
    msk_d = nc.dram_tensor("msk", [4, 128, 512], F32, kind="ExternalInput").ap()
    out_d = nc.dram_tensor("outT", [D, S], F32, kind="ExternalOutput").ap()

    with tile.TileContext(nc) as tc:
        with tc.tile_pool(name="persist", bufs=1) as pp, \
             tc.tile_pool(name="xin", bufs=3) as xp, \
             tc.tile_pool(name="etile", bufs=8) as ep, \
             tc.tile_pool(name="small", bufs=8) as sp, \
             tc.tile_pool(name="outtile", bufs=4) as op, \
             tc.tile_pool(name="psmm", bufs=4, space="PSUM") as ps_mm, \
             tc.tile_pool(name="psot", bufs=4, space="PSUM") as ps_ot:

            # ---- persistent SBUF tensors ----
            wq = pp.tile([128, 8, FG], BF16)   # [dp, do, f]
            wk = pp.tile([128, 8, FG], BF16)
            wv = pp.tile([128, 8, FG], BF16)
            wp = pp.tile([128, 4, D], BF16)    # [cp, co, j]
            qt = pp.tile([128, 4, S], BF16)    # [fp, fo, s]
            kt = pp.tile([128, 4, S], BF16)
            va = pp.tile([128, 16, 8, DH + 1], BF16)  # [skp, sko, h, dh|1]
            on_ = pp.tile([128, 4, S], BF16)   # renormed out^T  [cp, co, s]
            msk = pp.tile([128, 4, 512], BF16)
            bqs = pp.tile([128, 4], F32)
            bks = pp.tile([128, 4], F32)
            bvs = pp.tile([128, 4], F32)
            bps = pp.tile([128, 8], F32)

            nc.gpsimd.dma_start(wq[:], wqT_d.rearrange("(do dp) f -> dp do f", dp=128))
            nc.gpsimd.dma_start(wk[:], wkT_d.rearrange("(do dp) f -> dp do f", dp=128))
            nc.gpsimd.dma_start(wv[:], wvT_d.rearrange("(do dp) f -> dp do f", dp=128))
            nc.gpsimd.dma_start(wp[:], wpT_d.rearrange("(co cp) j -> cp co j", cp=128))
            nc.gpsimd.dma_start(msk[:], msk_d.rearrange("m p j -> p m j"))
            nc.sync.dma_start(bqs[:], bqs_d[:])
            nc.sync.dma_start(bks[:], bks_d[:])
            nc.sync.dma_start(bvs[:], bvs_d[:])
            nc.sync.dma_start(bps[:], bps_d[:])
            nc.vector.memset(va[:, :, :, DH:DH + 1], 1.0)

            xT_r = xT_d.rearrange("(do dp) s -> dp do s", dp=128)

            # ---- phase B: QKV projections, per 512-wide s block ----
            for sb in range(4):
                xblk = xp.tile([128, 8, 512], BF16)
                nc.gpsimd.dma_start(xblk[:], xT_r[:, :, sb * 512:(sb + 1) * 512])
                # Q^T and K^T: out[f=128, s=512], lhsT = w tile, rhs = x
                for w_sb, dst, bias, scl in ((wq, qt, bqs, SCALE), (wk, kt, bks, 1.0)):
                    for ft in range(4):
                        psq = ps_mm.tile([128, 512], F32, space="PSUM", tag="mm")
                        for do in range(8):
                            nc.tensor.matmul(
                                psq[:], w_sb[:, do, ft * 128:(ft + 1) * 128],
                                xblk[:, do, :],
                                start=(do == 0), stop=(do == 7))
                        nc.scalar.activation(
                            dst[:, ft, sb * 512:(sb + 1) * 512], psq[:],
                            AF.Identity, bias=bias[:, ft:ft + 1], scale=scl)
                # V: out[s=128, f=512], lhsT = x tile, rhs = wv
                for st in range(4):
                    psv = ps_mm.tile([128, 512], F32, space="PSUM", tag="mm")
                    for do in range(8):
                        nc.tensor.matmul(
                            psv[:], xblk[:, do, st * 128:(st + 1) * 128],
                            wv[:, do, :],
                            start=(do == 0), stop=(do == 7))
                    nc.vector.tensor_copy(
                        va[:, sb * 4 + st, :, :DH],
                        psv[:].rearrange("p (h d) -> p h d", h=8))

            # ---- phase C: attention + output projection per 512-wide sq block ----
            def emit_proj(bb, jts):
                # output projection for sq block bb: out^T[j, sq]
                for jt in jts:
                    psj = ps_mm.tile([128, 512], F32, space="PSUM", tag="mm")
                    for co in range(4):
                        nc.tensor.matmul(
                            psj[:], wp[:, co, jt * 128:(jt + 1) * 128],
                            on_[:, co, bb * 512:(bb + 1) * 512],
                            start=(co == 0), stop=(co == 3))
                    ot_sb = op.tile([128, 512], F32, tag="o")
                    nc.scalar.activation(ot_sb[:], psj[:], AF.Identity,
                                         bias=bps[:, jt:jt + 1])
                    nc.sync.dma_start(
                        out_d[jt * 128:(jt + 1) * 128, bb * 512:(bb + 1) * 512],
                        ot_sb[:])

            for b in range(4):
                nt = 4 * b + 4
                for p in range(4):  # head pairs (2p, 2p+1)
                    if b >= 1:  # interleave prev block's projection (dense PE work)
                        emit_proj(b - 1, [2 * p, 2 * p + 1])
                    ot0 = ps_ot.tile([DH + 1, 512], F32, space="PSUM", tag="ot")
                    ot1 = ps_ot.tile([DH + 1, 512], F32, space="PSUM", tag="ot")
                    for t in range(nt):
                        ksl = slice(t * 128, (t + 1) * 128)
                        qsl = slice(b * 512, (b + 1) * 512)
                        s0 = ps_mm.tile([128, 512], F32, space="PSUM", tag="mm")
                        s1 = ps_mm.tile([128, 512], F32, space="PSUM", tag="mm")
                        nc.tensor.matmul(s0[:], kt[0:64, p, ksl], qt[0:64, p, qsl],
                                         start=True, stop=True)
                        nc.tensor.matmul(s1[:], kt[64:128, p, ksl], qt[64:128, p, qsl],
                                         start=True, stop=True)
                        e0 = ep.tile([128, 512], BF16, tag="e")
                        e1 = ep.tile([128, 512], BF16, tag="e")
                        nc.scalar.activation(e0[:], s0[:], AF.Exp)
                        nc.scalar.activation(e1[:], s1[:], AF.Exp)
                        if t >= 4 * b:  # diagonal block: causal mask
                            m = t - 4 * b
                            nc.vector.tensor_tensor(e0[:], e0[:], msk[:, m, :],
                                                    mybir.AluOpType.mult)
                            nc.vector.tensor_tensor(e1[:], e1[:], msk[:, m, :],
                                                    mybir.AluOpType.mult)
                        nc.tensor.matmul(ot0[:], va[:, t, 2 * p, :], e0[:],
                                         start=(t == 0), stop=(t == nt - 1))
                        nc.tensor.matmul(ot1[:], va[:, t, 2 * p + 1, :], e1[:],
                                         start=(t == 0), stop=(t == nt - 1))
                    for h, otp in ((2 * p, ot0), (2 * p + 1, ot1)):
                        rec = sp.tile([1, 512], F32, tag="rec")
                        nc.vector.reciprocal(rec[:], otp[DH:DH + 1, :])
                        rb = sp.tile([DH, 512], F32, tag="rb")
                        nc.gpsimd.partition_broadcast(rb[:], rec[:])
                        r0 = 64 * (h % 2)
                        dst = on_[r0:r0 + 64, h // 2, b * 512:(b + 1) * 512]
                        nc.vector.tensor_tensor(dst, otp[0:DH, :], rb[:],
                                                mybir.AluOpType.mult)
                        nc.vector.tensor_scalar_add(dst, dst,
                                                    bvs[r0:r0 + 64, h // 2:h // 2 + 1])
            emit_proj(3, range(8))

    nc.compile()
    return nc


def kernel(x, Wq, bq, Wk, bk, Wv, bv, Wp, bp):
    global _NC
    if _NC is None:
        _NC = _build()

    x = np.asarray(x, np.float32)
    Wq, bq = np.asarray(Wq, np.float32), np.asarray(bq, np.float32)
    Wk, bk = np.asarray(Wk, np.float32), np.asarray(bk, np.float32)
    Wv, bv = np.asarray(Wv, np.float32), np.asarray(bv, np.float32)
    Wp, bp = np.asarray(Wp, np.float32), np.asarray(bp, np.float32)

    mask = np.zeros((4, 128, 512), np.float32)
    for m in range(4):
        i = np.arange(128)[:, None]
        j = np.arange(512)[None, :]
        mask[m] = (128 * m + i <= j).astype(np.float32)

    in_maps = []
    for c in range(N_CORES):
        b, g = c // 2, c % 2
        hs = range(8 * g, 8 * g + 8)
        kidx = np.array([dh * 16 + h for h in hs for dh in range(DH)])
        fsl = slice(FG * g, FG * (g + 1))
        bp_c = bp if g == 0 else np.zeros_like(bp)
        in_maps.append({
            "xT": np.ascontiguousarray(x[b].T),
            "wqT": np.ascontiguousarray(Wq[fsl].T),
            "wkT": np.ascontiguousarray(Wk[kidx].T),
            "wvT": np.ascontiguousarray(Wv[fsl].T),
            "wpT": np.ascontiguousarray(Wp[:, fsl].T),
            "bqs": np.ascontiguousarray((SCALE * bq[fsl]).reshape(4, 128).T),
            "bks": np.ascontiguousarray(bk[kidx].reshape(4, 128).T),
            "bvs": np.ascontiguousarray(bv[fsl].reshape(4, 128).T),
            "bps": np.ascontiguousarray(bp_c.reshape(8, 128).T),
            "msk": mask,
        })

    res = run_bass_kernel_spmd(_NC, in_maps, core_ids=list(range(N_CORES)))
    out = np.empty((B, S, D), np.float32)
    for b in range(B):
        acc = res.results[2 * b]["outT"] + res.results[2 * b + 1]["outT"]
        out[b] = acc.T
    return out



# revision 5
# speedup vs baseline: 1.3835x; 1.3835x over previous
"""Multi-head self-attention (B=4,S=2048,D=1024,H=16,DH=64, causal) on 8 trn2 cores.

Sharding: core c -> batch b=c//2, head-group g=c%2 (8 heads each).
Per-core pipeline (v2):
- QKV projections in bf16 (PE), streamed per 512-wide s block and software-
  pipelined against attention of the previous q-block.
- Scores via fp8e4m3 DoubleRow matmuls at 0.5 cycles/col: slot0 = q8*k8,
  slot1 = q8*dk8 where dk8 = fp8(k - k8) is a quantization-residual
  correction (halves the fp8 error; measured rel err ~8.5e-3).
- Causal mask added in PSUM by a small [-30*I] @ [c<p] matmul on the 128-wide
  diagonal strip; above-diagonal columns of diagonal tiles are skipped.
- exp on ACT engine batched over k-tile pairs ([128,1024] two-bank PSUM reads).
- attn@V in bf16 with ones-column denominator; renorm via
  reciprocal_approx_fast (DVE) + partition_broadcast (gpsimd) + mult (DVE).
- bv and bp folded host-side into the output-projection bias.
Host sums the two head-group partial outputs per batch.

K-projection quirk (reference views k as (B,S,DH,H)): head h uses Wk rows
[dh*16+h for dh in range(64)] -- handled by host-side row gather.
"""
import numpy as np
import ml_dtypes

import concourse.mybir as mybir
import concourse.tile as tile
from concourse import bacc
from concourse.bass_utils import run_bass_kernel_spmd

F32 = mybir.dt.float32
BF16 = mybir.dt.bfloat16
FP8 = mybir.dt.float8e4
AF = mybir.ActivationFunctionType
ALU = mybir.AluOpType
DR = mybir.MatmulPerfMode.DoubleRow

B, S, D, H, DH = 4, 2048, 1024, 16, 64
FG = 512          # features per head-group (8 heads * 64)
N_CORES = 8
SCALE = 0.125     # 1/sqrt(64)

_NC = None


def _build():
    nc = bacc.Bacc("TRN2", target_bir_lowering=False, debug=False,
                   num_devices=N_CORES, enable_asserts=False)
    xbT_d = nc.dram_tensor("xbT", [D, S], BF16, kind="ExternalInput").ap()
    wqT_d = nc.dram_tensor("wqT", [D, FG], BF16, kind="ExternalInput").ap()
    wkT_d = nc.dram_tensor("wkT", [D, FG], BF16, kind="ExternalInput").ap()
    wvT_d = nc.dram_tensor("wvT", [D, FG], BF16, kind="ExternalInput").ap()
    wpT_d = nc.dram_tensor("wpT", [FG, D], BF16, kind="ExternalInput").ap()
    bqs_d = nc.dram_tensor("bqs", [128, 4], F32, kind="ExternalInput").ap()
    bks_d = nc.dram_tensor("bks", [128, 4], F32, kind="ExternalInput").ap()
    bps_d = nc.dram_tensor("bps", [128, 8], F32, kind="ExternalInput").ap()
    negI_d = nc.dram_tensor("negI", [128, 128], BF16, kind="ExternalInput").ap()
    ltB_d = nc.dram_tensor("ltB", [128, 128], BF16, kind="ExternalInput").ap()
    out_d = nc.dram_tensor("outT", [D, S], F32, kind="ExternalOutput").ap()

    with tile.TileContext(nc) as tc:
        with tc.tile_pool(name="persist", bufs=1) as pp, \
             tc.tile_pool(name="xin", bufs=2) as xp, \
             tc.tile_pool(name="etile", bufs=4) as ep, \
             tc.tile_pool(name="small", bufs=4) as sp, \
             tc.tile_pool(name="outtile", bufs=3) as op, \
             tc.tile_pool(name="pspair", bufs=2, space="PSUM") as ps_pair, \
             tc.tile_pool(name="psproj", bufs=2, space="PSUM") as ps_proj, \
             tc.tile_pool(name="psot", bufs=2, space="PSUM") as ps_ot:

            # ---- persistent SBUF tensors ----
            wq = pp.tile([128, 8, FG], BF16)   # [dp, do, f]  (pre-scaled 1/8)
            wk = pp.tile([128, 8, FG], BF16)
            wv = pp.tile([128, 8, FG], BF16)
            wp = pp.tile([128, 4, D], BF16)    # [cp, co, j]
            qt8 = pp.tile([128, 4, S], FP8)    # [fp, fo, s]
            kt8 = pp.tile([128, 2, 4, S], FP8)  # [fp, slot(k8|dk8), fo, s]
            va = pp.tile([128, 16, 8, DH + 1], BF16)  # [skp, sko, h, dh|1]
            on_ = pp.tile([128, 4, S], BF16)   # renormed out^T  [cp, co, s]
            negI = pp.tile([128, 128], BF16)
            ltB = pp.tile([128, 128], BF16)
            bqs = pp.tile([128, 4], F32)
            bks = pp.tile([128, 4], F32)
            bps = pp.tile([128, 8], F32)

            nc.gpsimd.dma_start(wq[:], wqT_d.rearrange("(do dp) f -> dp do f", dp=128))
            nc.gpsimd.dma_start(wk[:], wkT_d.rearrange("(do dp) f -> dp do f", dp=128))
            nc.gpsimd.dma_start(wv[:], wvT_d.rearrange("(do dp) f -> dp do f", dp=128))
            nc.gpsimd.dma_start(wp[:], wpT_d.rearrange("(co cp) j -> cp co j", cp=128))
            nc.sync.dma_start(negI[:], negI_d[:])
            nc.sync.dma_start(ltB[:], ltB_d[:])
            nc.sync.dma_start(bqs[:], bqs_d[:])
            nc.sync.dma_start(bks[:], bks_d[:])
            nc.sync.dma_start(bps[:], bps_d[:])
            nc.vector.memset(va[:, :, :, DH:DH + 1], 1.0)

            xbT_r = xbT_d.rearrange("(do dp) s -> dp do s", dp=128)

            xtiles = {}

            def emit_xdma(sb):
                xb = xp.tile([128, 8, 512], BF16)
                nc.gpsimd.dma_start(xb[:], xbT_r[:, :, sb * 512:(sb + 1) * 512])
                xtiles[sb] = xb

            # ---- phase B: QKV projections for one 512-wide s block ----
            # group index: 0-3 Q(ft), 4-7 K(ft), 8-11 V(st)
            def emit_b_group(sb, gi):
                xb = xtiles[sb]
                ssl = slice(sb * 512, (sb + 1) * 512)
                if gi < 8:  # Q or K projection, output [f=128, s=512]
                    ft = gi % 4
                    w_sb = wq if gi < 4 else wk
                    ps = ps_proj.tile([128, 512], F32, space="PSUM", tag="proj")
                    for do in range(8):
                        nc.tensor.matmul(
                            ps[:], w_sb[:, do, ft * 128:(ft + 1) * 128],
                            xb[:, do, :], start=(do == 0), stop=(do == 7))
                    if gi < 4:
                        nc.vector.tensor_scalar_add(
                            qt8[:, ft, ssl], ps[:], bqs[:, ft:ft + 1])
                    else:
                        nc.vector.tensor_scalar_add(
                            kt8[:, 0, ft, ssl], ps[:], bks[:, ft:ft + 1])
                        # residual dk8 = (ps + bk) - k8   (fp8 round of both)
                        nc.vector.scalar_tensor_tensor(
                            kt8[:, 1, ft, ssl], ps[:], bks[:, ft:ft + 1],
                            kt8[:, 0, ft, ssl], ALU.add, ALU.subtract)
                else:       # V projection, output [s=128, f=512]
                    st = gi - 8
                    ps = ps_proj.tile([128, 512], F32, space="PSUM", tag="proj")
                    for do in range(8):
                        nc.tensor.matmul(
                            ps[:], xtiles[sb][:, do, st * 128:(st + 1) * 128],
                            wv[:, do, :], start=(do == 0), stop=(do == 7))
                    nc.vector.tensor_copy(
                        va[:, sb * 4 + st, :, :DH],
                        ps[:].rearrange("p (h d) -> p h d", h=8))

            # ---- fp8 DoubleRow score matmul for head h, k-tile t, q-block qb
            def mm_score(out_ap, h, t, qb, c0, start, stop):
                g2, j = h % 2, h // 2
                p0 = 64 * g2
                w = 512 - c0
                lhsT = kt8[p0:p0 + 64, :, j, 128 * t:128 * t + 128]
                qs = qt8[p0:p0 + 64, j, qb * 512 + c0:(qb + 1) * 512]
                rhs = qs.unsqueeze(1).broadcast_to([64, 2, w])
                nc.tensor.matmul(out_ap, lhsT, rhs, start=start, stop=stop,
                                 perf_mode=DR)

            # ---- attention for (q-block qb, head h) ----
            def emit_c(qb, h):
                nt = 4 * qb + 4
                qsl = slice(qb * 512, (qb + 1) * 512)
                ot = ps_ot.tile([DH + 1, 512], F32, space="PSUM", tag="ot")
                for u in range(nt // 2):
                    pt = ps_pair.tile([128, 1024], F32, space="PSUM", tag="pair")
                    et = ep.tile([128, 1024], BF16, tag="e")
                    for half in range(2):
                        t = 2 * u + half
                        m = t - 4 * qb
                        hsl = 512 * half
                        if m < 0:  # full tile
                            mm_score(pt[:, hsl:hsl + 512], h, t, qb, 0,
                                     True, True)
                        else:      # diagonal tile: trim cols, add mask strip
                            c0 = 128 * m
                            mm_score(pt[:, hsl + c0:hsl + 512], h, t, qb, c0,
                                     True, False)
                            nc.tensor.matmul(
                                pt[:, hsl + c0:hsl + c0 + 128], negI[:],
                                ltB[:], start=False, stop=True,
                                skip_group_check=True)
                    # exp (ACT), batched over the pair when both halves full
                    m0 = 2 * u - 4 * qb
                    if m0 < 0:
                        nc.scalar.activation(et[:], pt[:], AF.Exp)
                    else:
                        c0a, c0b = 128 * m0, 128 * (m0 + 1)
                        nc.scalar.activation(
                            et[:, c0a:512], pt[:, c0a:512], AF.Exp)
                        nc.scalar.activation(
                            et[:, 512 + c0b:1024], pt[:, 512 + c0b:1024],
                            AF.Exp)
                    for half in range(2):
                        t = 2 * u + half
                        m = t - 4 * qb
                        c0 = 0 if m < 0 else 128 * m
                        hsl = 512 * half
                        nc.tensor.matmul(
                            ot[0:DH + 1, c0:512], va[:, t, h, :],
                            et[:, hsl + c0:hsl + 512],
                            start=(t == 0), stop=(t == nt - 1),
                            skip_group_check=True)
                # softmax renorm: divide by ones-column row of ot
                dn = sp.tile([1, 512], F32, tag="dn")
                nc.vector.tensor_copy(dn[:], ot[DH:DH + 1, :])
                rec = sp.tile([1, 512], F32, tag="rec")
                nc.vector.reciprocal_approx_fast(rec[:], dn[:])
                rb = sp.tile([DH, 512], F32, tag="rb")
                nc.gpsimd.partition_broadcast(rb[:], rec[:])
                r0 = 64 * (h % 2)
                dst = on_[r0:r0 + 64, h // 2, qsl]
                nc.vector.tensor_tensor(dst, ot[0:DH, :], rb[:], ALU.mult)

            # ---- output projection for q-block qb: out^T[j, sq] ----
            def emit_proj(qb, jts):
                for jt in jts:
                    psj = ps_proj.tile([128, 512], F32, space="PSUM", tag="proj")
                    for co in range(4):
                        nc.tensor.matmul(
                            psj[:], wp[:, co, jt * 128:(jt + 1) * 128],
                            on_[:, co, qb * 512:(qb + 1) * 512],
                            start=(co == 0), stop=(co == 3))
                    ot_sb = op.tile([128, 512], F32, tag="o")
                    nc.vector.tensor_scalar_add(ot_sb[:], psj[:],
                                                bps[:, jt:jt + 1])
                    nc.sync.dma_start(
                        out_d[jt * 128:(jt + 1) * 128,
                              qb * 512:(qb + 1) * 512],
                        ot_sb[:])

            # ---- emission: software-pipeline B(qb+1) and proj(qb-1) into
            # the attention loop over (qb, h) ----
            emit_xdma(0)
            for gi in range(12):
                emit_b_group(0, gi)
            for qb in range(4):
                if qb < 3:
                    emit_xdma(qb + 1)
                for h in range(8):
                    if qb < 3:
                        for gi in range(12 * h // 8, 12 * (h + 1) // 8):
                            emit_b_group(qb + 1, gi)
                    emit_c(qb, h)
                    if qb >= 1:
                        emit_proj(qb - 1, [h])
            emit_proj(3, range(8))

    nc.compile()
    return nc


def kernel(x, Wq, bq, Wk, bk, Wv, bv, Wp, bp):
    global _NC
    if _NC is None:
        _NC = _build()

    x = np.asarray(x, np.float32)
    Wq, bq = np.asarray(Wq, np.float32), np.asarray(bq, np.float32)
    Wk, bk = np.asarray(Wk, np.float32), np.asarray(bk, np.float32)
    Wv, bv = np.asarray(Wv, np.float32), np.asarray(bv, np.float32)
    Wp, bp = np.asarray(Wp, np.float32), np.asarray(bp, np.float32)

    bf = ml_dtypes.bfloat16
    negI = np.ascontiguousarray((-30.0 * np.eye(128, dtype=np.float32)).astype(bf))
    i_ = np.arange(128)
    ltB = np.ascontiguousarray(
        (i_[None, :] < i_[:, None]).astype(np.float32).astype(bf))

    xbT = [np.ascontiguousarray(x[b].T.astype(bf)) for b in range(B)]

    in_maps = []
    for c in range(N_CORES):
        b, g = c // 2, c % 2
        hs = range(8 * g, 8 * g + 8)
        kidx = np.array([dh * 16 + h for h in hs for dh in range(DH)])
        fsl = slice(FG * g, FG * (g + 1))
        bp_c = (bp if g == 0 else 0.0) + Wp[:, fsl] @ bv[fsl]
        in_maps.append({
            "xbT": xbT[b],
            "wqT": np.ascontiguousarray((SCALE * Wq[fsl].T).astype(bf)),
            "wkT": np.ascontiguousarray(Wk[kidx].T.astype(bf)),
            "wvT": np.ascontiguousarray(Wv[fsl].T.astype(bf)),
            "wpT": np.ascontiguousarray(Wp[:, fsl].T.astype(bf)),
            "bqs": np.ascontiguousarray((SCALE * bq[fsl]).reshape(4, 128).T),
            "bks": np.ascontiguousarray(bk[kidx].reshape(4, 128).T),
            "bps": np.ascontiguousarray(bp_c.reshape(8, 128).T.astype(np.float32)),
            "negI": negI,
            "ltB": ltB,
        })

    res = run_bass_kernel_spmd(_NC, in_maps, core_ids=list(range(N_CORES)))
    out = np.empty((B, S, D), np.float32)
    for b in range(B):
        acc = res.results[2 * b]["outT"] + res.results[2 * b + 1]["outT"]
        out[b] = acc.T
    return out


# revision 9
# speedup vs baseline: 1.4207x; 1.0269x over previous
"""Multi-head self-attention (B=4,S=2048,D=1024,H=16,DH=64, causal) on 8 trn2 cores.

Sharding: core c -> batch b=c//2, head-group g=c%2 (8 heads each).
Per-core pipeline (v2):
- QKV projections in bf16 (PE), streamed per 512-wide s block and software-
  pipelined against attention of the previous q-block.
- Scores in bf16 (fp8 DoubleRow was tried and measured SLOWER: fp8 matmuls
  pay ~150-210ns PE mode-switch transition penalties that eat the 2x
  column-rate gain).
- Causal mask added in PSUM by a small [-30*I] @ [c<p] matmul on the 128-wide
  diagonal strip; above-diagonal columns of diagonal tiles are skipped.
- exp on ACT engine batched over k-tile pairs ([128,1024] two-bank PSUM reads).
- attn@V in bf16 with ones-column denominator; renorm via
  reciprocal_approx_fast (DVE) + partition_broadcast (gpsimd) + mult (DVE).
- bv and bp folded host-side into the output-projection bias.
Host sums the two head-group partial outputs per batch.

K-projection quirk (reference views k as (B,S,DH,H)): head h uses Wk rows
[dh*16+h for dh in range(64)] -- handled by host-side row gather.
"""
import numpy as np
import ml_dtypes

import concourse.mybir as mybir
import concourse.tile as tile
from concourse import bacc
from concourse.bass_utils import run_bass_kernel_spmd

F32 = mybir.dt.float32
BF16 = mybir.dt.bfloat16
FP8 = mybir.dt.float8e4
AF = mybir.ActivationFunctionType
ALU = mybir.AluOpType
DR = mybir.MatmulPerfMode.DoubleRow

B, S, D, H, DH = 4, 2048, 1024, 16, 64
FG = 512          # features per head-group (8 heads * 64)
N_CORES = 8
SCALE = 0.125     # 1/sqrt(64)

_NC = None


def _build():
    nc = bacc.Bacc("TRN2", target_bir_lowering=False, debug=False,
                   num_devices=N_CORES, enable_asserts=False)
    xbT_d = nc.dram_tensor("xbT", [D, S], BF16, kind="ExternalInput").ap()
    wqT_d = nc.dram_tensor("wqT", [D, FG], BF16, kind="ExternalInput").ap()
    wkT_d = nc.dram_tensor("wkT", [D, FG], BF16, kind="ExternalInput").ap()
    wvT_d = nc.dram_tensor("wvT", [D, FG], BF16, kind="ExternalInput").ap()
    wpT_d = nc.dram_tensor("wpT", [FG, D], BF16, kind="ExternalInput").ap()
    bqs_d = nc.dram_tensor("bqs", [128, 4], F32, kind="ExternalInput").ap()
    bks_d = nc.dram_tensor("bks", [128, 4], F32, kind="ExternalInput").ap()
    bps_d = nc.dram_tensor("bps", [128, 8], F32, kind="ExternalInput").ap()
    negI_d = nc.dram_tensor("negI", [128, 128], BF16, kind="ExternalInput").ap()
    ltB_d = nc.dram_tensor("ltB", [128, 128], BF16, kind="ExternalInput").ap()
    out_d = nc.dram_tensor("outT", [D, S], F32, kind="ExternalOutput").ap()

    with tile.TileContext(nc) as tc:
        with tc.tile_pool(name="persist", bufs=1) as pp, \
             tc.tile_pool(name="xin", bufs=2) as xp, \
             tc.tile_pool(name="etile", bufs=4) as ep, \
             tc.tile_pool(name="small", bufs=4) as sp, \
             tc.tile_pool(name="outtile", bufs=3) as op, \
             tc.tile_pool(name="pspair", bufs=2, space="PSUM") as ps_pair, \
             tc.tile_pool(name="psproj", bufs=2, space="PSUM") as ps_proj, \
             tc.tile_pool(name="psot", bufs=2, space="PSUM") as ps_ot:

            # ---- persistent SBUF tensors ----
            wq = pp.tile([128, 8, FG], BF16)   # [dp, do, f]  (pre-scaled 1/8)
            wk = pp.tile([128, 8, FG], BF16)
            wv = pp.tile([128, 8, FG], BF16)
            wp = pp.tile([128, 4, D], BF16)    # [cp, co, j]
            qt = pp.tile([128, 4, S], BF16)    # [fp, fo, s]
            kt = pp.tile([128, 4, S], BF16)
            va = pp.tile([128, 16, 8, DH + 1], BF16)  # [skp, sko, h, dh|1]
            on_ = pp.tile([128, 4, S], BF16)   # renormed out^T  [cp, co, s]
            negI = pp.tile([128, 128], BF16)
            ltB = pp.tile([128, 128], BF16)
            bqs = pp.tile([128, 4], F32)
            bks = pp.tile([128, 4], F32)
            bps = pp.tile([128, 8], F32)

            nc.gpsimd.dma_start(wq[:], wqT_d.rearrange("(do dp) f -> dp do f", dp=128))
            nc.gpsimd.dma_start(wk[:], wkT_d.rearrange("(do dp) f -> dp do f", dp=128))
            nc.gpsimd.dma_start(wv[:], wvT_d.rearrange("(do dp) f -> dp do f", dp=128))
            nc.gpsimd.dma_start(wp[:], wpT_d.rearrange("(co cp) j -> cp co j", cp=128))
            nc.sync.dma_start(negI[:], negI_d[:])
            nc.sync.dma_start(ltB[:], ltB_d[:])
            nc.sync.dma_start(bqs[:], bqs_d[:])
            nc.sync.dma_start(bks[:], bks_d[:])
            nc.sync.dma_start(bps[:], bps_d[:])
            nc.vector.memset(va[:, :, :, DH:DH + 1], 1.0)

            xbT_r = xbT_d.rearrange("(do dp) s -> dp do s", dp=128)

            xtiles = {}

            def emit_xdma(sb):
                xb = xp.tile([128, 8, 512], BF16)
                nc.gpsimd.dma_start(xb[:], xbT_r[:, :, sb * 512:(sb + 1) * 512])
                xtiles[sb] = xb

            # ---- phase B: QKV projections for one 512-wide s block ----
            # group index: 0-3 Q(ft), 4-7 K(ft), 8-11 V(st)
            def emit_b_group(sb, gi):
                xb = xtiles[sb]
                ssl = slice(sb * 512, (sb + 1) * 512)
                if gi < 8:  # Q or K projection, output [f=128, s=512]
                    ft = gi % 4
                    w_sb = wq if gi < 4 else wk
                    ps = ps_proj.tile([128, 512], F32, space="PSUM", tag="proj")
                    for do in range(8):
                        nc.tensor.matmul(
                            ps[:], w_sb[:, do, ft * 128:(ft + 1) * 128],
                            xb[:, do, :], start=(do == 0), stop=(do == 7))
                    dst, bias = (qt, bqs) if gi < 4 else (kt, bks)
                    nc.vector.tensor_scalar_add(
                        dst[:, ft, ssl], ps[:], bias[:, ft:ft + 1])
                else:       # V projection, output [s=128, f=512]
                    st = gi - 8
                    ps = ps_proj.tile([128, 512], F32, space="PSUM", tag="proj")
                    for do in range(8):
                        nc.tensor.matmul(
                            ps[:], xtiles[sb][:, do, st * 128:(st + 1) * 128],
                            wv[:, do, :], start=(do == 0), stop=(do == 7))
                    nc.vector.tensor_copy(
                        va[:, sb * 4 + st, :, :DH],
                        ps[:].rearrange("p (h d) -> p h d", h=8))

            # ---- bf16 score matmul for head h, k-tile t, q-block qb ----
            def mm_score(out_ap, h, t, qb, c0, start, stop):
                g2, j = h % 2, h // 2
                p0 = 64 * g2
                lhsT = kt[p0:p0 + 64, j, 128 * t:128 * t + 128]
                rhs = qt[p0:p0 + 64, j, qb * 512 + c0:(qb + 1) * 512]
                nc.tensor.matmul(out_ap, lhsT, rhs, start=start, stop=stop)

            # ---- attention for (q-block qb, head h) ----
            def emit_c(qb, h):
                nt = 4 * qb + 4
                qsl = slice(qb * 512, (qb + 1) * 512)
                ot = ps_ot.tile([DH + 1, 512], F32, space="PSUM", tag="ot")
                for u in range(nt // 2):
                    pt = ps_pair.tile([128, 1024], F32, space="PSUM", tag="pair")
                    et = ep.tile([128, 1024], BF16, tag="e")
                    for half in range(2):
                        t = 2 * u + half
                        m = t - 4 * qb
                        hsl = 512 * half
                        if m < 0:  # full tile
                            mm_score(pt[:, hsl:hsl + 512], h, t, qb, 0,
                                     True, True)
                        else:      # diagonal tile: trim cols, add mask strip
                            c0 = 128 * m
                            mm_score(pt[:, hsl + c0:hsl + 512], h, t, qb, c0,
                                     True, False)
                            nc.tensor.matmul(
                                pt[:, hsl + c0:hsl + c0 + 128], negI[:],
                                ltB[:], start=False, stop=True,
                                skip_group_check=True)
                    # exp (ACT), batched over the pair when both halves full
                    m0 = 2 * u - 4 * qb
                    if m0 < 0:
                        nc.scalar.activation(et[:], pt[:], AF.Exp)
                    else:
                        c0a, c0b = 128 * m0, 128 * (m0 + 1)
                        nc.scalar.activation(
                            et[:, c0a:512], pt[:, c0a:512], AF.Exp)
                        nc.scalar.activation(
                            et[:, 512 + c0b:1024], pt[:, 512 + c0b:1024],
                            AF.Exp)
                    for half in range(2):
                        t = 2 * u + half
                        m = t - 4 * qb
                        c0 = 0 if m < 0 else 128 * m
                        hsl = 512 * half
                        nc.tensor.matmul(
                            ot[0:DH + 1, c0:512], va[:, t, h, :],
                            et[:, hsl + c0:hsl + 512],
                            start=(t == 0), stop=(t == nt - 1),
                            skip_group_check=True)
                # softmax renorm: divide by ones-column row of ot
                dn = sp.tile([1, 512], F32, tag="dn")
                nc.vector.tensor_copy(dn[:], ot[DH:DH + 1, :])
                rec = sp.tile([1, 512], F32, tag="rec")
                nc.vector.reciprocal_approx_fast(rec[:], dn[:])
                rb = sp.tile([DH, 512], F32, tag="rb")
                nc.gpsimd.partition_broadcast(rb[:], rec[:])
                r0 = 64 * (h % 2)
                dst = on_[r0:r0 + 64, h // 2, qsl]
                nc.vector.tensor_tensor(dst, ot[0:DH, :], rb[:], ALU.mult)

            # ---- output projection for q-block qb: out^T[j, sq] ----
            def emit_proj(qb, jts):
                for jt in jts:
                    psj = ps_proj.tile([128, 512], F32, space="PSUM", tag="proj")
                    for co in range(4):
                        nc.tensor.matmul(
                            psj[:], wp[:, co, jt * 128:(jt + 1) * 128],
                            on_[:, co, qb * 512:(qb + 1) * 512],
                            start=(co == 0), stop=(co == 3))
                    ot_sb = op.tile([128, 512], F32, tag="o")
                    nc.vector.tensor_scalar_add(ot_sb[:], psj[:],
                                                bps[:, jt:jt + 1])
                    nc.sync.dma_start(
                        out_d[jt * 128:(jt + 1) * 128,
                              qb * 512:(qb + 1) * 512],
                        ot_sb[:])

            # ---- emission: software-pipeline B(qb+1) and proj(qb-1) into
            # the attention loop over (qb, h) ----
            emit_xdma(0)
            for gi in range(12):
                emit_b_group(0, gi)
            for qb in range(4):
                if qb < 3:
                    emit_xdma(qb + 1)
                for h in range(8):
                    if qb < 3:
                        for gi in range(12 * h // 8, 12 * (h + 1) // 8):
                            emit_b_group(qb + 1, gi)
                    emit_c(qb, h)
                    if qb >= 1:
                        emit_proj(qb - 1, [h])
            emit_proj(3, range(8))

    nc.compile()
    return nc


def kernel(x, Wq, bq, Wk, bk, Wv, bv, Wp, bp):
    global _NC
    if _NC is None:
        _NC = _build()

    x = np.asarray(x, np.float32)
    Wq, bq = np.asarray(Wq, np.float32), np.asarray(bq, np.float32)
    Wk, bk = np.asarray(Wk, np.float32), np.asarray(bk, np.float32)
    Wv, bv = np.asarray(Wv, np.float32), np.asarray(bv, np.float32)
    Wp, bp = np.asarray(Wp, np.float32), np.asarray(bp, np.float32)

    bf = ml_dtypes.bfloat16
    negI = np.ascontiguousarray((-30.0 * np.eye(128, dtype=np.float32)).astype(bf))
    i_ = np.arange(128)
    ltB = np.ascontiguousarray(
        (i_[None, :] < i_[:, None]).astype(np.float32).astype(bf))

    xbT = [np.ascontiguousarray(x[b].T.astype(bf)) for b in range(B)]

    in_maps = []
    for c in range(N_CORES):
        b, g = c // 2, c % 2
        hs = range(8 * g, 8 * g + 8)
        kidx = np.array([dh * 16 + h for h in hs for dh in range(DH)])
        fsl = slice(FG * g, FG * (g + 1))
        bp_c = (bp if g == 0 else 0.0) + Wp[:, fsl] @ bv[fsl]
        in_maps.append({
            "xbT": xbT[b],
            "wqT": np.ascontiguousarray((SCALE * Wq[fsl].T).astype(bf)),
            "wkT": np.ascontiguousarray(Wk[kidx].T.astype(bf)),
            "wvT": np.ascontiguousarray(Wv[fsl].T.astype(bf)),
            "wpT": np.ascontiguousarray(Wp[:, fsl].T.astype(bf)),
            "bqs": np.ascontiguousarray((SCALE * bq[fsl]).reshape(4, 128).T),
            "bks": np.ascontiguousarray(bk[kidx].reshape(4, 128).T),
            "bps": np.ascontiguousarray(bp_c.reshape(8, 128).T.astype(np.float32)),
            "negI": negI,
            "ltB": ltB,
        })

    res = run_bass_kernel_spmd(_NC, in_maps, core_ids=list(range(N_CORES)))
    out = np.empty((B, S, D), np.float32)
    for b in range(B):
        acc = res.results[2 * b]["outT"] + res.results[2 * b + 1]["outT"]
        out[b] = acc.T
    return out
